# revision 17
# baseline (speedup 1.0000x reference)
"""Trainium2 Bass kernel for nn_Concat_73607149519362.

Math (decomposed concat-MLP attention score):
    score[b, d, e] = dec[b, d] @ w_dec + enc[b, e] @ w_enc + bias

Sharding: data-parallel over batch, 32 batches / 8 cores = 4 per core.

Per-core design (v15):
  - enc is *pre-transposed on the host* (dim on partitions, packed
    [p, j, e]) so the enc projection is a PE matmul over the partition
    axis: 8 accumulating steps x 2 psum halves -> eproj [1, 1024] row.
    To balance HBM bytes vs DMA-engine time it is shipped split:
    dim-tiles j=0..1 as int8 via SWDGE cast-DMA on the gpsimd ring,
    j=2..7 as fp16 via the sync/HWDGE ring - both rings stream in
    parallel.
  - dec is int8 (x32) natural layout; DVE scalar_tensor_tensor
    (int8 x fp16-weight, fp32 accum) -> dproj columns directly.
  - eproj + bias -> f16 enc_row on ACT; PE broadcasts it to a
    [128, 1024] PSUM tile (f16 ones outer product).
  - Output builds out = sat_i8(ebc + dproj_col) are split: chunks
    t=0,1 on ACT, t=2,3 on DVE (both saturate on int8, HW verified),
    tracked with per-batch semaphores.
  - Output int8 (scaled by s_out = 127/(5*sigma_w+|b|)); host gather
    divides by s_out.

HBM traffic/core: enc 1+6MB + dec 2MB + out 2MB ~ 11.25MB.
"""

import os
from contextlib import ExitStack

os.environ.setdefault("JAX_PLATFORMS", "axon")

import numpy as np

import concourse.bass as bass
import concourse.mybir as mybir
from concourse.bass_utils import run_bass_kernel_spmd

B, DEC, ENC, DIM = 32, 512, 1024, 1024
NCORES = 8
BPC = B // NCORES  # batches per core

F32 = mybir.dt.float32
F16 = mybir.dt.float16
I8 = mybir.dt.int8
P = 128
TE = DIM // P  # 8 enc dim-tiles (j)
J8 = 2  # dim-tiles shipped int8 (cast DMA); TE-J8 shipped f16
TD = DEC // P  # 4 dec 128-row chunks
NBLK = 512
NB = ENC // NBLK  # 2

S_IN = 32.0  # input int8 quantization scale
K_SIG = 5.0  # output int8 range in units of sigma_w
OUT_I8 = True


def _enc8_groups(b):
    if b == 0:
        return [(0, 1), (1, J8)]
    return [(0, J8)]


def _enc16_groups(b):
    if b == 0:
        return [(J8, 4), (4, TE)]
    if b == BPC - 1:
        return [(J8, 6), (6, 7), (7, TE)]
    return [(J8, TE)]


def _dec_groups(b):
    if b == BPC - 1:
        return [(0, TD - 1), (TD - 1, TD)]
    return [(0, TD)]


def _build(out_i8=OUT_I8):
    nc = bass.Bass("TRN2")
    odt = I8 if out_i8 else F16
    enc8_h = nc.dram_tensor("encT8", [BPC * P, J8 * ENC], I8, kind="ExternalInput")
    enc16_h = nc.dram_tensor(
        "encT16", [BPC * P, (TE - J8) * ENC], F16, kind="ExternalInput"
    )
    dec_h = nc.dram_tensor("dec_q", [BPC * DEC, DIM], I8, kind="ExternalInput")
    wencT_h = nc.dram_tensor("w_encT", [P, TE], F16, kind="ExternalInput")
    wdec_h = nc.dram_tensor("w_dec", [P, DIM], F16, kind="ExternalInput")
    bias_h = nc.dram_tensor("bias", [1, 1], F32, kind="ExternalInput")
    ones_h = nc.dram_tensor("ones_in", [1, P], F16, kind="ExternalInput")
    out_h = nc.dram_tensor(
        "out", [(BPC - 1) * DEC, ENC], odt, kind="ExternalOutput"
    )
    out3_h = nc.dram_tensor("out_b3", [DEC, ENC], F16, kind="ExternalOutput")

    enc8_r = enc8_h.ap().rearrange("(b p) x -> b p x", p=P)
    enc16_r = enc16_h.ap().rearrange("(b p) x -> b p x", p=P)
    dec_r = dec_h.ap().rearrange("(b p t) d -> b p (t d)", p=P, t=TD)
    out_r = out_h.ap().rearrange("(b p t) e -> b p (t e)", p=P, t=TD)
    out3_r = out3_h.ap().rearrange("(p t) e -> p (t e)", p=P, t=TD)

    with ExitStack() as ctx:

        def sb(name, shape, dt=F32):
            return ctx.enter_context(nc.sbuf_tensor(name, shape, dt))

        encT_t = [sb(f"encT{i}", [P, TE, ENC], F16) for i in range(BPC)]
        dec_t = [sb(f"dec{i}", [P, TD, DIM], I8) for i in range(BPC)]
        out_t = [
            sb(f"out{i}", [P, TD, ENC], odt if i < BPC - 1 else F16)
            for i in range(BPC)
        ]
        w_encT = sb("w_encT_b", [P, TE], F16)
        w_dec_b = sb("w_dec_b", [P, DIM], F16)
        ones_row = sb("ones_row", [1, P], F16)
        bias_b = sb("bias_b", [1, 1])
        enc_row = [sb(f"enc_row{i}", [1, ENC], F16) for i in range(BPC)]
        dproj = [sb(f"dproj{i}", [P, TD]) for i in range(BPC)]
        scr = sb("scr", [P, DIM], F16)

        pe_enc = [
            ctx.enter_context(nc.psum_tensor(f"pe_enc{i}", [1, ENC], F32))
            for i in range(2)
        ]
        ebc = [
            ctx.enter_context(nc.psum_tensor(f"ebc{i}", [P, ENC], F32))
            for i in range(2)
        ]

        s_misc = ctx.enter_context(nc.semaphore(name="s_misc"))
        s_w = ctx.enter_context(nc.semaphore(name="s_w"))
        s_enc8 = [
            [
                ctx.enter_context(nc.semaphore(name=f"s_enc8_{b}g{g}"))
                for g in range(len(_enc8_groups(b)))
            ]
            for b in range(BPC)
        ]
        s_enc16 = [
            [
                ctx.enter_context(nc.semaphore(name=f"s_enc16_{b}g{g}"))
                for g in range(len(_enc16_groups(b)))
            ]
            for b in range(BPC)
        ]
        s_dec = [
            [
                ctx.enter_context(nc.semaphore(name=f"s_dec{b}g{g}"))
                for g in range(len(_dec_groups(b)))
            ]
            for b in range(BPC)
        ]
        s_eproj = ctx.enter_context(nc.semaphore(name="s_eproj"))
        s_rowa = ctx.enter_context(nc.semaphore(name="s_rowa"))
        s_rowd = ctx.enter_context(nc.semaphore(name="s_rowd"))
        s_ebc = ctx.enter_context(nc.semaphore(name="s_ebc"))
        s_dp = [
            ctx.enter_context(nc.semaphore(name=f"s_dp{b}")) for b in range(BPC)
        ]
        # per-batch build-completion sems; last batch split by engine
        s_bb = [
            ctx.enter_context(nc.semaphore(name=f"s_bb{b}")) for b in range(BPC - 1)
        ]
        s_b3a = ctx.enter_context(nc.semaphore(name="s_b3a"))
        s_b3d = ctx.enter_context(nc.semaphore(name="s_b3d"))
        s_out = ctx.enter_context(nc.semaphore(name="s_out"))

        def enc_sb2d(b):
            return encT_t[b].ap().rearrange("p j r -> p (j r)")

        with nc.Block(no_gpsimd_drain=True) as block:

            @block.sync
            def _(sync):
                sync.dma_start(w_dec_b.ap(), wdec_h.ap()).then_inc(s_w, 16)
                for b in range(BPC):
                    for g, (lo, hi) in enumerate(_dec_groups(b)):
                        sync.dma_start(
                            dec_t[b].ap().rearrange("p t d -> p (t d)")[
                                :, lo * DIM : hi * DIM
                            ],
                            dec_r[b][:, lo * DIM : hi * DIM],
                        ).then_inc(s_dec[b][g], 16)
                for b in range(BPC):
                    for g, (lo, hi) in enumerate(_enc16_groups(b)):
                        sync.dma_start(
                            enc_sb2d(b)[:, lo * ENC : hi * ENC],
                            enc16_r[b][:, (lo - J8) * ENC : (hi - J8) * ENC],
                        ).then_inc(s_enc16[b][g], 16)
                for b in range(BPC - 1):
                    sync.wait_ge(s_bb[b], TD)
                    sync.dma_start(
                        out_r[b],
                        out_t[b].ap().rearrange("p t e -> p (t e)"),
                    ).then_inc(s_out, 16)
                b = BPC - 1
                for t in (2, 3, 0, 1):
                    if t >= 2:
                        sync.wait_ge(s_b3d, t - 1)
                    else:
                        sync.wait_ge(s_b3a, t + 1)
                    sync.dma_start(
                        out3_r[:, t * ENC : (t + 1) * ENC],
                        out_t[b].ap().rearrange("p t e -> p (t e)")[
                            :, t * ENC : (t + 1) * ENC
                        ],
                    ).then_inc(s_out, 16)
                # ensure every output byte is in HBM before block teardown
                sync.wait_ge(s_out, (BPC - 1 + TD) * 16)

            @block.gpsimd
            def _(gpsimd):
                for b in range(BPC):
                    for g, (lo, hi) in enumerate(_enc8_groups(b)):
                        # SWDGE cast DMA: int8 DRAM -> fp16 SBUF
                        gpsimd.dma_start(
                            enc_sb2d(b)[:, lo * ENC : hi * ENC],
                            enc8_r[b][:, lo * ENC : hi * ENC],
                        ).then_inc(s_enc8[b][g], 16)

            @block.tensor
            def _(pe):
                def enc_mms(b):
                    if b == 0:
                        pe.wait_ge(s_misc, 16)  # w_encT
                    if b >= 2:
                        # pe_enc[b%2] free once batch b-2's enc_row read it
                        pe.wait_ge(s_rowa, b - 1)
                    lasti = None
                    j_order = list(range(J8, TE)) + list(range(J8))
                    for jj, j in enumerate(j_order):
                        for g, (lo, hi) in enumerate(_enc8_groups(b)):
                            if j == lo:
                                pe.wait_ge(s_enc8[b][g], 16)
                        for g, (lo, hi) in enumerate(_enc16_groups(b)):
                            if j == lo:
                                pe.wait_ge(s_enc16[b][g], 16)
                        for h in range(NB):
                            lasti = nc.tensor.matmul(
                                pe_enc[b % 2].ap()[0:1, h * NBLK : (h + 1) * NBLK],
                                w_encT.ap()[:, j : j + 1],
                                encT_t[b].ap()[:, j, h * NBLK : (h + 1) * NBLK],
                                start=(jj == 0),
                                stop=(jj == TE - 1),
                            )
                    lasti.then_inc(s_eproj, 1)

                def ebc_mms(b):
                    if b == 0:
                        pe.wait_ge(s_misc, 32)  # ones_row
                    if b < 2:
                        pe.wait_ge(s_rowa, b + 1)
                    else:
                        pe.wait_ge(s_rowd, b - 1)
                    if b >= 2:
                        # ebc[b%2] free once batch b-2's builds consumed it
                        pe.wait_ge(s_bb[b - 2], TD)
                    lasti = None
                    for h in range(NB):
                        lasti = nc.tensor.matmul(
                            ebc[b % 2].ap()[:, h * NBLK : (h + 1) * NBLK],
                            ones_row.ap(),
                            enc_row[b].ap()[0:1, h * NBLK : (h + 1) * NBLK],
                            start=True,
                            stop=True,
                        )
                    lasti.then_inc(s_ebc, 1)

                enc_mms(0)
                ebc_mms(0)
                enc_mms(1)
                ebc_mms(1)
                enc_mms(2)
                enc_mms(3)
                ebc_mms(2)
                ebc_mms(3)

            @block.vector
            def _(vector):
                vector.wait_ge(s_w, 16)
                for b in range(BPC):
                    for t in range(TD):
                        for g, (lo, hi) in enumerate(_dec_groups(b)):
                            if t == lo:
                                vector.wait_ge(s_dec[b][g], 16)
                        nc.vector.scalar_tensor_tensor(
                            out=scr.ap(),
                            in0=dec_t[b].ap()[:, t, :],
                            scalar=1.0,
                            in1=w_dec_b.ap(),
                            op0=mybir.AluOpType.mult,
                            op1=mybir.AluOpType.mult,
                            accum_out=dproj[b].ap()[:, t : t + 1],
                        ).then_inc(s_dp[b], 1)
                for b in (2, 3):
                    vector.wait_ge(s_misc, 48)
                    vector.wait_ge(s_eproj, b + 1)
                    nc.vector.tensor_scalar(
                        out=enc_row[b].ap(),
                        in0=pe_enc[b % 2].ap(),
                        scalar1=bias_b.ap()[0:1, 0:1],
                        scalar2=None,
                        op0=mybir.AluOpType.add,
                    ).then_inc(s_rowd, 1)
                b = BPC - 1
                vector.wait_ge(s_ebc, BPC)
                for t in (2, 3):
                    nc.vector.tensor_scalar(
                        out=out_t[b].ap()[:, t, :],
                        in0=ebc[b % 2].ap(),
                        scalar1=dproj[b].ap()[:, t : t + 1],
                        scalar2=None,
                        op0=mybir.AluOpType.add,
                    ).then_inc(s_b3d, 1)

            @block.scalar
            def _(scalar):
                scalar.dma_start(w_encT.ap(), wencT_h.ap()).then_inc(s_misc, 16)
                scalar.dma_start(ones_row.ap(), ones_h.ap()).then_inc(s_misc, 16)
                scalar.dma_start(bias_b.ap(), bias_h.ap()).then_inc(s_misc, 16)
                for b in range(BPC):
                    if b < 2:
                        scalar.wait_ge(s_eproj, b + 1)
                        nc.scalar.activation(
                            enc_row[b].ap(),
                            pe_enc[b % 2].ap(),
                            mybir.ActivationFunctionType.Identity,
                            bias=bias_b.ap()[0:1, 0:1],
                        ).then_inc(s_rowa, 1)
                    scalar.wait_ge(s_ebc, b + 1)
                    for t in range(TD) if b < BPC - 1 else (0, 1):
                        scalar.wait_ge(s_dp[b], t + 1)
                        bld = nc.scalar.add(
                            out_t[b].ap()[:, t, :],
                            ebc[b % 2].ap(),
                            add=dproj[b].ap()[:, t : t + 1],
                        )
                        if b < BPC - 1:
                            bld.then_inc(s_bb[b], 1)
                        else:
                            bld.then_inc(s_b3a, 1)

    return nc


_NC_CACHE = {}
_STATE = {"s_out": 1.0}


def _get_nc():
    if "nc" not in _NC_CACHE:
        _NC_CACHE["nc"] = _build()
    return _NC_CACHE["nc"]


def _shard_inputs(decoder_states, encoder_states, mlp_weight, mlp_bias):
    dec = np.asarray(decoder_states, dtype=np.float32)
    enc = np.asarray(encoder_states, dtype=np.float32)
    w = np.asarray(mlp_weight, dtype=np.float32).reshape(2 * DIM)
    bias = float(np.asarray(mlp_bias, dtype=np.float32).reshape(1)[0])
    w_enc, w_dec = w[:DIM], w[DIM:]

    if OUT_I8:
        sigw = float(np.sqrt((w_enc**2).sum() + (w_dec**2).sum()))
        s_out = 127.0 / (K_SIG * sigw + abs(bias) + 1e-12)
    else:
        s_out = 1.0
    _STATE["s_out"] = s_out

    dec_q = np.clip(np.rint(dec * S_IN), -127, 127).astype(np.int8)
    # transposed enc [B, p, j, e]
    encT = enc.transpose(0, 2, 1).reshape(B, TE, P, ENC).transpose(0, 2, 1, 3)
    enc8 = np.clip(np.rint(encT[:, :, :J8, :] * S_IN), -127, 127).astype(np.int8)
    enc16 = encT[:, :, J8:, :].astype(np.float16)

    # int8 dim-tiles' weights absorb the 1/S_IN dequant; all scaled by s_out
    wt = (w_enc * s_out).reshape(TE, P).T.astype(np.float32).copy()  # [p, j]
    wt[:, :J8] /= S_IN
    wencT = np.ascontiguousarray(wt.astype(np.float16))
    wdec_dev = np.ascontiguousarray(
        np.tile((w_dec * (s_out / S_IN)).astype(np.float16).reshape(1, DIM), (P, 1))
    )
    bias_dev = np.array([[bias * s_out]], dtype=np.float32)
    ones = np.ones((1, P), dtype=np.float16)

    in_maps = []
    for i in range(NCORES):
        lo = i * BPC
        in_maps.append(
            {
                "encT8": np.ascontiguousarray(
                    enc8[lo : lo + BPC].reshape(BPC * P, J8 * ENC)
                ),
                "encT16": np.ascontiguousarray(
                    enc16[lo : lo + BPC].reshape(BPC * P, (TE - J8) * ENC)
                ),
                "dec_q": np.ascontiguousarray(
                    dec_q[lo : lo + BPC].reshape(BPC * DEC, DIM)
                ),
                "w_encT": wencT,
                "w_dec": wdec_dev,
                "bias": bias_dev,
                "ones_in": ones,
            }
        )
    return in_maps


def _gather(res):
    s_out = _STATE["s_out"]
    shards = []
    for r in res.results:
        a = r["out"].astype(np.float32).reshape(BPC - 1, DEC, ENC)
        b3 = r["out_b3"].astype(np.float32).reshape(1, DEC, ENC)
        shards.append(np.concatenate([a, b3], axis=0))
    out = np.concatenate(shards, axis=0)
    out /= s_out
    return out


def kernel(decoder_states, encoder_states, step, mlp_weight, mlp_bias, **_ignored):
    in_maps = _shard_inputs(decoder_states, encoder_states, mlp_weight, mlp_bias)
    res = run_bass_kernel_spmd(_get_nc(), in_maps, core_ids=list(range(NCORES)))
    return _gather(res)


# revision 18
# speedup vs baseline: 1.0618x; 1.0618x over previous
"""Trainium2 Bass kernel for nn_Concat_73607149519362.

Math (decomposed concat-MLP attention score):
    score[b, d, e] = dec[b, d] @ w_dec + enc[b, e] @ w_enc + bias

Sharding: data-parallel over batch, 32 batches / 8 cores = 4 per core.

Per-core design (v15):
  - enc is *pre-transposed on the host* (dim on partitions, packed
    [p, j, e]) so the enc projection is a PE matmul over the partition
    axis: 8 accumulating steps x 2 psum halves -> eproj [1, 1024] row.
    To balance HBM bytes vs DMA-engine time it is shipped split:
    dim-tiles j=0..1 as int8 via SWDGE cast-DMA on the gpsimd ring,
    j=2..7 as fp16 via the sync/HWDGE ring - both rings stream in
    parallel.
  - dec is int8 (x32) natural layout; DVE scalar_tensor_tensor
    (int8 x fp16-weight, fp32 accum) -> dproj columns directly.
  - eproj + bias -> f16 enc_row on ACT; PE broadcasts it to a
    [128, 1024] PSUM tile (f16 ones outer product).
  - Output builds out = sat_i8(ebc + dproj_col) are split: chunks
    t=0,1 on ACT, t=2,3 on DVE (both saturate on int8, HW verified),
    tracked with per-batch semaphores.
  - Output int8 (scaled by s_out = 127/(5*sigma_w+|b|)); host gather
    divides by s_out.

HBM traffic/core: enc 1+6MB + dec 2MB + out 2MB ~ 11.25MB.
"""

import os
from contextlib import ExitStack

os.environ.setdefault("JAX_PLATFORMS", "axon")

import numpy as np

import concourse.bass as bass
import concourse.mybir as mybir
from concourse.bass_utils import run_bass_kernel_spmd

B, DEC, ENC, DIM = 32, 512, 1024, 1024
NCORES = 8
BPC = B // NCORES  # batches per core

F32 = mybir.dt.float32
F16 = mybir.dt.float16
I8 = mybir.dt.int8
P = 128
TE = DIM // P  # 8 enc dim-tiles (j)
J8 = 2  # dim-tiles shipped int8 (cast DMA); TE-J8 shipped f16
TD = DEC // P  # 4 dec 128-row chunks
NBLK = 512
NB = ENC // NBLK  # 2

S_IN = 32.0  # input int8 quantization scale
K_SIG = 5.0  # output int8 range in units of sigma_w
OUT_I8 = True


def _enc8_groups(b):
    if b == 0:
        return [(0, 1), (1, J8)]
    return [(0, J8)]


def _enc16_groups(b):
    if b == 0:
        return [(J8, 4), (4, TE)]
    if b == BPC - 1:
        return [(J8, 6), (6, 7), (7, TE)]
    return [(J8, TE)]


def _dec_groups(b):
    if b == BPC - 1:
        return [(0, TD - 1), (TD - 1, TD)]
    return [(0, TD)]


def _build(out_i8=OUT_I8):
    nc = bass.Bass("TRN2")
    odt = I8 if out_i8 else F16
    enc8_h = nc.dram_tensor("encT8", [BPC * P, J8 * ENC], I8, kind="ExternalInput")
    enc16_h = nc.dram_tensor(
        "encT16", [BPC * P, (TE - J8) * ENC], F16, kind="ExternalInput"
    )
    dec_h = nc.dram_tensor("dec_q", [BPC * DEC, DIM], I8, kind="ExternalInput")
    wencT_h = nc.dram_tensor("w_encT", [P, TE], F16, kind="ExternalInput")
    wdec_h = nc.dram_tensor("w_dec", [P, DIM], F16, kind="ExternalInput")
    bias_h = nc.dram_tensor("bias", [1, 1], F32, kind="ExternalInput")
    ones_h = nc.dram_tensor("ones_in", [1, P], F16, kind="ExternalInput")
    out_h = nc.dram_tensor(
        "out", [(BPC - 1) * DEC, ENC], odt, kind="ExternalOutput"
    )
    out3_h = nc.dram_tensor("out_b3", [DEC, ENC], F16, kind="ExternalOutput")

    enc8_r = enc8_h.ap().rearrange("(b p) x -> b p x", p=P)
    enc16_r = enc16_h.ap().rearrange("(b p) x -> b p x", p=P)
    dec_r = dec_h.ap().rearrange("(b p t) d -> b p (t d)", p=P, t=TD)
    out_r = out_h.ap().rearrange("(b p t) e -> b p (t e)", p=P, t=TD)
    out3_r = out3_h.ap().rearrange("(p t) e -> p (t e)", p=P, t=TD)

    with ExitStack() as ctx:

        def sb(name, shape, dt=F32):
            return ctx.enter_context(nc.sbuf_tensor(name, shape, dt))

        encT_t = [sb(f"encT{i}", [P, TE, ENC], F16) for i in range(BPC)]
        dec_t = [sb(f"dec{i}", [P, TD, DIM], I8) for i in range(BPC)]
        out_t = [
            sb(f"out{i}", [P, TD, ENC], odt if i < BPC - 1 else F16)
            for i in range(BPC)
        ]
        w_encT = sb("w_encT_b", [P, TE], F16)
        w_dec_b = sb("w_dec_b", [P, DIM], F16)
        ones_row = sb("ones_row", [1, P], F16)
        bias_b = sb("bias_b", [1, 1])
        enc_row = [sb(f"enc_row{i}", [1, ENC], F16) for i in range(BPC)]
        dproj = [sb(f"dproj{i}", [P, TD]) for i in range(BPC)]
        scr = sb("scr", [P, DIM], F16)

        pe_enc = [
            ctx.enter_context(nc.psum_tensor(f"pe_enc{i}", [1, ENC], F32))
            for i in range(2)
        ]
        ebc = [
            ctx.enter_context(nc.psum_tensor(f"ebc{i}", [P, ENC], F32))
            for i in range(2)
        ]

        s_misc = ctx.enter_context(nc.semaphore(name="s_misc"))
        s_w = ctx.enter_context(nc.semaphore(name="s_w"))
        s_enc8 = [
            [
                ctx.enter_context(nc.semaphore(name=f"s_enc8_{b}g{g}"))
                for g in range(len(_enc8_groups(b)))
            ]
            for b in range(BPC)
        ]
        s_enc16 = [
            [
                ctx.enter_context(nc.semaphore(name=f"s_enc16_{b}g{g}"))
                for g in range(len(_enc16_groups(b)))
            ]
            for b in range(BPC)
        ]
        s_dec = [
            [
                ctx.enter_context(nc.semaphore(name=f"s_dec{b}g{g}"))
                for g in range(len(_dec_groups(b)))
            ]
            for b in range(BPC)
        ]
        s_eproj = ctx.enter_context(nc.semaphore(name="s_eproj"))
        s_rowa = ctx.enter_context(nc.semaphore(name="s_rowa"))
        s_rowd = ctx.enter_context(nc.semaphore(name="s_rowd"))
        s_ebc = ctx.enter_context(nc.semaphore(name="s_ebc"))
        s_dp = [
            ctx.enter_context(nc.semaphore(name=f"s_dp{b}")) for b in range(BPC)
        ]
        # per-batch build-completion sems; last batch split by engine
        s_bb = [
            ctx.enter_context(nc.semaphore(name=f"s_bb{b}")) for b in range(BPC - 1)
        ]
        s_b3a = ctx.enter_context(nc.semaphore(name="s_b3a"))
        s_b3d = ctx.enter_context(nc.semaphore(name="s_b3d"))
        s_out = ctx.enter_context(nc.semaphore(name="s_out"))

        def enc_sb2d(b):
            return encT_t[b].ap().rearrange("p j r -> p (j r)")

        with nc.Block(no_gpsimd_drain=True) as block:

            @block.sync
            def _(sync):
                sync.dma_start(w_dec_b.ap(), wdec_h.ap()).then_inc(s_w, 16)
                def dec_dma(b):
                    for g, (lo, hi) in enumerate(_dec_groups(b)):
                        sync.dma_start(
                            dec_t[b].ap().rearrange("p t d -> p (t d)")[
                                :, lo * DIM : hi * DIM
                            ],
                            dec_r[b][:, lo * DIM : hi * DIM],
                        ).then_inc(s_dec[b][g], 16)

                def enc16_dma(b):
                    for g, (lo, hi) in enumerate(_enc16_groups(b)):
                        sync.dma_start(
                            enc_sb2d(b)[:, lo * ENC : hi * ENC],
                            enc16_r[b][:, (lo - J8) * ENC : (hi - J8) * ENC],
                        ).then_inc(s_enc16[b][g], 16)

                dec_dma(0)
                dec_dma(BPC - 1)
                enc16_dma(0)
                dec_dma(1)
                enc16_dma(1)
                dec_dma(2)
                enc16_dma(2)
                enc16_dma(3)
                for b in range(BPC - 1):
                    sync.wait_ge(s_bb[b], TD)
                    sync.dma_start(
                        out_r[b],
                        out_t[b].ap().rearrange("p t e -> p (t e)"),
                    ).then_inc(s_out, 16)
                b = BPC - 1
                for t in (2, 3, 0, 1):
                    if t >= 2:
                        sync.wait_ge(s_b3d, t - 1)
                    else:
                        sync.wait_ge(s_b3a, t + 1)
                    sync.dma_start(
                        out3_r[:, t * ENC : (t + 1) * ENC],
                        out_t[b].ap().rearrange("p t e -> p (t e)")[
                            :, t * ENC : (t + 1) * ENC
                        ],
                    ).then_inc(s_out, 16)
                # ensure every output byte is in HBM before block teardown
                sync.wait_ge(s_out, (BPC - 1 + TD) * 16)

            @block.gpsimd
            def _(gpsimd):
                for b in range(BPC):
                    for g, (lo, hi) in enumerate(_enc8_groups(b)):
                        # SWDGE cast DMA: int8 DRAM -> fp16 SBUF
                        gpsimd.dma_start(
                            enc_sb2d(b)[:, lo * ENC : hi * ENC],
                            enc8_r[b][:, lo * ENC : hi * ENC],
                        ).then_inc(s_enc8[b][g], 16)

            @block.tensor
            def _(pe):
                def enc_mms(b):
                    if b == 0:
                        pe.wait_ge(s_misc, 16)  # w_encT
                    if b >= 2:
                        # pe_enc[b%2] free once batch b-2's enc_row read it
                        pe.wait_ge(s_rowa, b - 1)
                    lasti = None
                    for j in range(TE):
                        for g, (lo, hi) in enumerate(_enc8_groups(b)):
                            if j == lo:
                                pe.wait_ge(s_enc8[b][g], 16)
                        for g, (lo, hi) in enumerate(_enc16_groups(b)):
                            if j == lo:
                                pe.wait_ge(s_enc16[b][g], 16)
                        for h in range(NB):
                            lasti = nc.tensor.matmul(
                                pe_enc[b % 2].ap()[0:1, h * NBLK : (h + 1) * NBLK],
                                w_encT.ap()[:, j : j + 1],
                                encT_t[b].ap()[:, j, h * NBLK : (h + 1) * NBLK],
                                start=(j == 0),
                                stop=(j == TE - 1),
                            )
                    lasti.then_inc(s_eproj, 1)

                def ebc_mms(b):
                    if b == 0:
                        pe.wait_ge(s_misc, 32)  # ones_row
                    if b < 2:
                        pe.wait_ge(s_rowa, b + 1)
                    else:
                        pe.wait_ge(s_rowd, b - 1)
                    if b >= 2:
                        # ebc[b%2] free once batch b-2's builds consumed it
                        pe.wait_ge(s_bb[b - 2], TD)
                    lasti = None
                    for h in range(NB):
                        lasti = nc.tensor.matmul(
                            ebc[b % 2].ap()[:, h * NBLK : (h + 1) * NBLK],
                            ones_row.ap(),
                            enc_row[b].ap()[0:1, h * NBLK : (h + 1) * NBLK],
                            start=True,
                            stop=True,
                        )
                    lasti.then_inc(s_ebc, 1)

                enc_mms(0)
                ebc_mms(0)
                enc_mms(1)
                ebc_mms(1)
                enc_mms(2)
                enc_mms(3)
                ebc_mms(2)
                ebc_mms(3)

            @block.vector
            def _(vector):
                vector.wait_ge(s_w, 16)
                for b in range(BPC):
                    for t in range(TD):
                        for g, (lo, hi) in enumerate(_dec_groups(b)):
                            if t == lo:
                                vector.wait_ge(s_dec[b][g], 16)
                        nc.vector.scalar_tensor_tensor(
                            out=scr.ap(),
                            in0=dec_t[b].ap()[:, t, :],
                            scalar=1.0,
                            in1=w_dec_b.ap(),
                            op0=mybir.AluOpType.mult,
                            op1=mybir.AluOpType.mult,
                            accum_out=dproj[b].ap()[:, t : t + 1],
                        ).then_inc(s_dp[b], 1)
                for b in (2, 3):
                    vector.wait_ge(s_misc, 48)
                    vector.wait_ge(s_eproj, b + 1)
                    nc.vector.tensor_scalar(
                        out=enc_row[b].ap(),
                        in0=pe_enc[b % 2].ap(),
                        scalar1=bias_b.ap()[0:1, 0:1],
                        scalar2=None,
                        op0=mybir.AluOpType.add,
                    ).then_inc(s_rowd, 1)
                b = BPC - 1
                vector.wait_ge(s_ebc, BPC)
                for t in (2, 3):
                    nc.vector.tensor_scalar(
                        out=out_t[b].ap()[:, t, :],
                        in0=ebc[b % 2].ap(),
                        scalar1=dproj[b].ap()[:, t : t + 1],
                        scalar2=None,
                        op0=mybir.AluOpType.add,
                    ).then_inc(s_b3d, 1)

            @block.scalar
            def _(scalar):
                scalar.dma_start(w_encT.ap(), wencT_h.ap()).then_inc(s_misc, 16)
                scalar.dma_start(ones_row.ap(), ones_h.ap()).then_inc(s_misc, 16)
                scalar.dma_start(bias_b.ap(), bias_h.ap()).then_inc(s_misc, 16)
                for b in range(BPC):
                    if b < 2:
                        scalar.wait_ge(s_eproj, b + 1)
                        nc.scalar.activation(
                            enc_row[b].ap(),
                            pe_enc[b % 2].ap(),
                            mybir.ActivationFunctionType.Identity,
                            bias=bias_b.ap()[0:1, 0:1],
                        ).then_inc(s_rowa, 1)
                    scalar.wait_ge(s_ebc, b + 1)
                    for t in range(TD) if b < BPC - 1 else (0, 1):
                        scalar.wait_ge(s_dp[b], t + 1)
                        bld = nc.scalar.add(
                            out_t[b].ap()[:, t, :],
                            ebc[b % 2].ap(),
                            add=dproj[b].ap()[:, t : t + 1],
                        )
                        if b < BPC - 1:
                            bld.then_inc(s_bb[b], 1)
                        else:
                            bld.then_inc(s_b3a, 1)

    return nc


_NC_CACHE = {}
_STATE = {"s_out": 1.0}


def _get_nc():
    if "nc" not in _NC_CACHE:
        _NC_CACHE["nc"] = _build()
    return _NC_CACHE["nc"]


def _shard_inputs(decoder_states, encoder_states, mlp_weight, mlp_bias):
    dec = np.asarray(decoder_states, dtype=np.float32)
    enc = np.asarray(encoder_states, dtype=np.float32)
    w = np.asarray(mlp_weight, dtype=np.float32).reshape(2 * DIM)
    bias = float(np.asarray(mlp_bias, dtype=np.float32).reshape(1)[0])
    w_enc, w_dec = w[:DIM], w[DIM:]

    if OUT_I8:
        sigw = float(np.sqrt((w_enc**2).sum() + (w_dec**2).sum()))
        s_out = 127.0 / (K_SIG * sigw + abs(bias) + 1e-12)
    else:
        s_out = 1.0
    _STATE["s_out"] = s_out

    dec_q = np.clip(np.rint(dec * S_IN), -127, 127).astype(np.int8)
    # transposed enc [B, p, j, e]
    encT = enc.transpose(0, 2, 1).reshape(B, TE, P, ENC).transpose(0, 2, 1, 3)
    enc8 = np.clip(np.rint(encT[:, :, :J8, :] * S_IN), -127, 127).astype(np.int8)
    enc16 = encT[:, :, J8:, :].astype(np.float16)

    # int8 dim-tiles' weights absorb the 1/S_IN dequant; all scaled by s_out
    wt = (w_enc * s_out).reshape(TE, P).T.astype(np.float32).copy()  # [p, j]
    wt[:, :J8] /= S_IN
    wencT = np.ascontiguousarray(wt.astype(np.float16))
    wdec_dev = np.ascontiguousarray(
        np.tile((w_dec * (s_out / S_IN)).astype(np.float16).reshape(1, DIM), (P, 1))
    )
    bias_dev = np.array([[bias * s_out]], dtype=np.float32)
    ones = np.ones((1, P), dtype=np.float16)

    in_maps = []
    for i in range(NCORES):
        lo = i * BPC
        in_maps.append(
            {
                "encT8": np.ascontiguousarray(
                    enc8[lo : lo + BPC].reshape(BPC * P, J8 * ENC)
                ),
                "encT16": np.ascontiguousarray(
                    enc16[lo : lo + BPC].reshape(BPC * P, (TE - J8) * ENC)
                ),
                "dec_q": np.ascontiguousarray(
                    dec_q[lo : lo + BPC].reshape(BPC * DEC, DIM)
                ),
                "w_encT": wencT,
                "w_dec": wdec_dev,
                "bias": bias_dev,
                "ones_in": ones,
            }
        )
    return in_maps


def _gather(res):
    s_out = _STATE["s_out"]
    shards = []
    for r in res.results:
        a = r["out"].astype(np.float32).reshape(BPC - 1, DEC, ENC)
        b3 = r["out_b3"].astype(np.float32).reshape(1, DEC, ENC)
        shards.append(np.concatenate([a, b3], axis=0))
    out = np.concatenate(shards, axis=0)
    out /= s_out
    return out


def kernel(decoder_states, encoder_states, step, mlp_weight, mlp_bias, **_ignored):
    in_maps = _shard_inputs(decoder_states, encoder_states, mlp_weight, mlp_bias)
    res = run_bass_kernel_spmd(_get_nc(), in_maps, core_ids=list(range(NCORES)))
    return _gather(res)
